# revision 28
# baseline (speedup 1.0000x reference)
"""Box-from-mask kernel for Trainium2 (8 NeuronCores, SPMD data-parallel).

Problem: masks [100, 800, 1280] f32 -> boxes [100, 2, 2] f32 where
box[n] = [[xmin, ymin], [xmax, ymax]] of {(y, x) : masks[n, y, x] > 0.5},
with empty-mask sentinels xmin=W, ymin=H, xmax=-1, ymax=-1.

Sharding: N axis padded 100 -> 104 = 8 cores x 13 masks; each core computes
its boxes independently (no communication).

Per-core device pipeline, per [128, 1280] row-tile of each mask:
  - DVE tensor_scalar(is_gt 0.5) -> 0/1 bf16 tile, with accum_out(max)
    giving per-row "any" in one pass (2x_2P perf mode).
  - PE ones-matmul over the binary tile accumulates per-column counts
    in PSUM across the mask's 7 row-tiles.
Row/col "any" vectors are turned into min/max indices with the masked-max
trick (max of any*(D - idx) and any*(idx + 1)) via fused tensor_tensor_reduce,
a 7-step cross-partition max fold, and a tiny host-side affine fixup.
"""

import sys

for _p in ("/opt/trn_rl_repo", "/opt/pypackages"):
    if _p not in sys.path:
        sys.path.append(_p)

import ml_dtypes
import numpy as np

import concourse.bass as bass
import concourse.tile as tile
from concourse import bacc, mybir
from concourse.bass_utils import run_bass_kernel_spmd

N, H, W = 100, 800, 1280
N_CORES = 8
K = 13  # masks per core (8 * 13 = 104 >= 100, zero-padded)
THRESHOLD = 0.5

fp32 = mybir.dt.float32
fp16 = mybir.dt.float16
bf16 = mybir.dt.bfloat16
Op = mybir.AluOpType


def _row_tiles(h):
    """[(row0, nrows), ...] covering h rows in 128-row tiles."""
    return [(r, min(128, h - r)) for r in range(0, h, 128)]


def _chunks(w):
    """[(col0, ncols), ...] covering w cols in <=512-col chunks (PSUM bank)."""
    return [(c, min(512, w - c)) for c in range(0, w, 512)]


def _ntp(h):
    """Columns per mask in the row-any tile: tile count padded to even."""
    nt = len(_row_tiles(h))
    return nt + (nt & 1)


def build_program(k=K, h=H, w=W):
    """One-core Bass/Tile program; run SPMD on all 8 cores."""
    tiles = _row_tiles(h)
    chunks = _chunks(w)
    ntp = _ntp(h)
    nt = len(tiles)

    nc = bacc.Bacc(
        "TRN2", target_bir_lowering=False, debug=False, enable_asserts=False
    )
    masks = nc.dram_tensor("masks", [k, h, w], fp32, kind="ExternalInput").ap()
    c1r = nc.dram_tensor("c1r", [128, ntp], fp16, kind="ExternalInput").ap()
    c2r = nc.dram_tensor("c2r", [128, ntp], fp16, kind="ExternalInput").ap()
    c1x = nc.dram_tensor("c1x", [k, w], fp16, kind="ExternalInput").ap()
    c2x = nc.dram_tensor("c2x", [k, w], fp16, kind="ExternalInput").ap()
    # ohs[p, c] = 1 iff c == k-1: sliding window ohs[:, k-1-j : 2k-1-j] is a
    # [128, k] one-hot-column matrix selecting PSUM output partition j.
    ohs = nc.dram_tensor("ohs", [128, 2 * k], bf16, kind="ExternalInput").ap()
    rows_out = nc.dram_tensor(
        "rows_out", [128, 2 * k], fp16, kind="ExternalOutput"
    ).ap()
    cols_out = nc.dram_tensor("cols_out", [k, 2], fp16, kind="ExternalOutput").ap()

    # PSUM accumulation is split into two mask groups so the first group's
    # cols reduction overlaps the second group's main loop instead of all
    # landing in a serial tail after the last matmul.
    g0 = (k + 1) // 2
    groups = [(0, g0)] + ([(g0, k - g0)] if k > g0 else [])
    # DMA row-tiles in pairs (1.25MB per transfer) for fewer, larger DMAs
    pairs = [(t, min(2, nt - t)) for t in range(0, nt, 2)]

    with tile.TileContext(nc) as tc:
        with (
            tc.tile_pool(name="raw", bufs=5) as rawp,
            tc.tile_pool(name="bin", bufs=6) as binp,
            tc.tile_pool(name="consts", bufs=1) as constp,
            tc.tile_pool(name="small", bufs=2) as smallp,
            tc.tile_pool(name="psum", bufs=1, space="PSUM") as psump,
        ):
            # consts ride gpsimd SWDGE queues so SP's HWDGE queues start
            # streaming mask tiles immediately
            c1r_t = constp.tile([128, ntp], fp16)
            nc.gpsimd.dma_start(c1r_t[:], c1r)
            c2r_t = constp.tile([128, ntp], fp16)
            nc.gpsimd.dma_start(c2r_t[:], c2r)
            c1x_t = constp.tile([k, w], fp16)
            nc.gpsimd.dma_start(c1x_t[:], c1x)
            c2x_t = constp.tile([k, w], fp16)
            nc.gpsimd.dma_start(c2x_t[:], c2x)
            ohs_t = constp.tile([128, 2 * k], bf16)
            nc.gpsimd.dma_start(ohs_t[:], ohs)

            rowany = constp.tile([128, k * ntp], fp32)
            nc.gpsimd.memset(rowany[:], 0.0)
            negh = constp.tile([128, 1], fp32)
            nc.gpsimd.memset(negh[:], -THRESHOLD)
            rr = constp.tile([128, 2 * k], fp16)
            cc = [
                [
                    psump.tile([gsz, cw], fp32, name=f"cc{g}_{ci}", tag=f"cc{g}_{ci}")
                    for ci, (_, cw) in enumerate(chunks)
                ]
                for g, (_, gsz) in enumerate(groups)
            ]

            def emit_cols_tail(g, gstart, gsz):
                """(count > 0) * iota per chunk, then max-reduce along X."""
                csc1 = constp.tile([gsz, w], fp16, name=f"csc1_{g}")
                csc2 = constp.tile([gsz, w], fp16, name=f"csc2_{g}")
                cr = constp.tile([gsz, 2], fp16, name=f"cr_{g}")
                for ci, (c0, cw) in enumerate(chunks):
                    nc.vector.scalar_tensor_tensor(
                        out=csc1[:, c0 : c0 + cw],
                        in0=cc[g][ci][:, :],
                        scalar=0.0,
                        in1=c1x_t[0:gsz, c0 : c0 + cw],
                        op0=Op.is_gt,
                        op1=Op.mult,
                    )
                    nc.vector.scalar_tensor_tensor(
                        out=csc2[:, c0 : c0 + cw],
                        in0=cc[g][ci][:, :],
                        scalar=0.0,
                        in1=c2x_t[0:gsz, c0 : c0 + cw],
                        op0=Op.is_gt,
                        op1=Op.mult,
                    )
                nc.vector.tensor_reduce(
                    out=cr[:, 0:1], in_=csc1[:], axis=mybir.AxisListType.X, op=Op.max
                )
                nc.vector.tensor_reduce(
                    out=cr[:, 1:2], in_=csc2[:], axis=mybir.AxisListType.X, op=Op.max
                )
                nc.sync.dma_start(cols_out[gstart : gstart + gsz, :], cr[:])

            for g, (gstart, gsz) in enumerate(groups):
                for jl in range(gsz):
                    j = gstart + jl
                    for tp, npair in pairs:
                        r0 = tiles[tp][0]
                        prows = sum(tiles[tp + i][1] for i in range(npair))
                        raw = rawp.tile([128, 2 * w], fp32)
                        if npair == 2 and prows == 256:
                            nc.sync.dma_start(
                                raw[:, : 2 * w].rearrange("p (a x) -> p a x", a=2),
                                masks[j, r0 : r0 + 256, :].rearrange(
                                    "(a p) x -> p a x", p=128
                                ),
                            )
                        else:
                            nc.sync.dma_start(
                                raw[: tiles[tp][1], :w],
                                masks[j, r0 : r0 + tiles[tp][1], :],
                            )
                            if npair == 2:
                                nc.sync.dma_start(
                                    raw[: tiles[tp + 1][1], w : 2 * w],
                                    masks[j, r0 + 128 : r0 + 128 + tiles[tp + 1][1], :],
                                )
                        for i in range(npair):
                            t = tp + i
                            nr = tiles[t][1]
                            rv = raw[:nr, i * w : i * w + w]
                            b = binp.tile([128, w], bf16)
                            acc = rowany[:nr, j * ntp + t : j * ntp + t + 1]
                            if (j * nt + t) % 2 == 0:
                                # DVE: binary = (x > 0.5), accum(max) = row-any
                                nc.vector.tensor_scalar(
                                    out=b[:nr, :],
                                    in0=rv,
                                    scalar1=THRESHOLD,
                                    scalar2=None,
                                    op0=Op.is_gt,
                                    op1=Op.max,
                                    accum_out=acc,
                                )
                            else:
                                # ACT: relu(x-0.5) > 0 iff x > 0.5 (exact);
                                # accum(sum of non-negatives) > 0 iff row-any
                                nc.scalar.activation(
                                    out=b[:nr, :],
                                    in_=rv,
                                    func=mybir.ActivationFunctionType.Relu,
                                    bias=negh[:nr, :],
                                    scale=1.0,
                                    accum_out=acc,
                                )
                            for ci, (c0, cw) in enumerate(chunks):
                                nc.tensor.matmul(
                                    cc[g][ci][:, :],
                                    ohs_t[:nr, k - 1 - jl : k - 1 - jl + gsz],
                                    b[:nr, c0 : c0 + cw],
                                    start=(jl == 0 and t == 0),
                                    stop=(jl == gsz - 1 and t == nt - 1),
                                )
                    # rows tail: rr[:, 2j] = max_t((any>0) * (H - idx)),
                    #            rr[:, 2j+1] = max_t((any>0) * (idx + 1))
                    sc1 = smallp.tile([128, ntp], fp16, tag="sc1")
                    nc.vector.scalar_tensor_tensor(
                        out=sc1[:],
                        in0=rowany[:, j * ntp : (j + 1) * ntp],
                        scalar=0.0,
                        in1=c1r_t[:],
                        op0=Op.is_gt,
                        op1=Op.mult,
                    )
                    nc.vector.tensor_reduce(
                        out=rr[:, 2 * j : 2 * j + 1],
                        in_=sc1[:],
                        axis=mybir.AxisListType.X,
                        op=Op.max,
                    )
                    sc2 = smallp.tile([128, ntp], fp16, tag="sc2")
                    nc.vector.scalar_tensor_tensor(
                        out=sc2[:],
                        in0=rowany[:, j * ntp : (j + 1) * ntp],
                        scalar=0.0,
                        in1=c2r_t[:],
                        op0=Op.is_gt,
                        op1=Op.mult,
                    )
                    nc.vector.tensor_reduce(
                        out=rr[:, 2 * j + 1 : 2 * j + 2],
                        in_=sc2[:],
                        axis=mybir.AxisListType.X,
                        op=Op.max,
                    )
                emit_cols_tail(g, gstart, gsz)

            # the 128-partition max fold of rr happens host-side (6.6KB/core)
            nc.sync.dma_start(rows_out, rr[:])

    nc.compile()
    return nc


def make_consts(k=K, h=H, w=W):
    tiles = _row_tiles(h)
    ntp = _ntp(h)
    p = np.arange(128)

    c1r = np.zeros((128, ntp), np.float16)
    c2r = np.zeros((128, ntp), np.float16)
    for t, (r0, nr) in enumerate(tiles):
        idx = r0 + p
        valid = p < nr
        c1r[:, t] = np.where(valid, h - idx, 0)
        c2r[:, t] = np.where(valid, idx + 1, 0)

    x = np.arange(w)
    c1x = np.broadcast_to((w - x).astype(np.float16), (k, w)).copy()
    c2x = np.broadcast_to((x + 1).astype(np.float16), (k, w)).copy()
    ohs = np.zeros((128, 2 * k), ml_dtypes.bfloat16)
    ohs[:, k - 1] = 1
    return {"c1r": c1r, "c2r": c2r, "c1x": c1x, "c2x": c2x, "ohs": ohs}


def postprocess(results, k=K, h=H, w=W):
    """Per-core (rows_out, cols_out) -> boxes [n_cores * k, 2, 2] f32."""
    boxes = np.empty((len(results) * k, 2, 2), np.float32)
    for c, r in enumerate(results):
        rows = np.asarray(r["rows_out"], np.float32).max(axis=0)
        cols = np.asarray(r["cols_out"], np.float32).reshape(k, 2)
        sl = slice(c * k, (c + 1) * k)
        boxes[sl, 0, 0] = w - cols[:, 0]  # xmin
        boxes[sl, 0, 1] = h - rows[0::2]  # ymin
        boxes[sl, 1, 0] = cols[:, 1] - 1  # xmax
        boxes[sl, 1, 1] = rows[1::2] - 1  # ymax
    return boxes


_cache = {}


def _get_program():
    if "nc" not in _cache:
        _cache["nc"] = build_program()
        _cache["consts"] = make_consts()
    return _cache["nc"], _cache["consts"]


def make_in_maps(masks):
    """Pad masks to 104 and build the 8 per-core input maps."""
    masks = np.ascontiguousarray(np.asarray(masks, dtype=np.float32))
    _, consts = _get_program()
    n_pad = N_CORES * K
    if masks.shape[0] < n_pad:
        pad = np.zeros((n_pad - masks.shape[0], H, W), np.float32)
        masks = np.concatenate([masks, pad], axis=0)
    return [
        {"masks": masks[c * K : (c + 1) * K], **consts} for c in range(N_CORES)
    ]


def kernel(masks):
    nc, _ = _get_program()
    in_maps = make_in_maps(masks)
    res = run_bass_kernel_spmd(nc, in_maps, core_ids=list(range(N_CORES)))
    return postprocess(res.results)[:N]


# revision 31
# speedup vs baseline: 1.1443x; 1.1443x over previous
"""Box-from-mask kernel for Trainium2 (8 NeuronCores, SPMD data-parallel).

Problem: masks [100, 800, 1280] f32 -> boxes [100, 2, 2] f32 where
box[n] = [[xmin, ymin], [xmax, ymax]] of {(y, x) : masks[n, y, x] > 0.5},
with empty-mask sentinels xmin=W, ymin=H, xmax=-1, ymax=-1.

Sharding: N axis padded 100 -> 104 = 8 cores x 13 masks; each core computes
its boxes independently (no communication).

Per-core device pipeline, per [128, 1280] row-tile of each mask:
  - DVE tensor_scalar(is_gt 0.5) -> 0/1 bf16 tile, with accum_out(max)
    giving per-row "any" in one pass (2x_2P perf mode).
  - PE ones-matmul over the binary tile accumulates per-column counts
    in PSUM across the mask's 7 row-tiles.
Row/col "any" vectors are turned into min/max indices with the masked-max
trick (max of any*(D - idx) and any*(idx + 1)) via fused tensor_tensor_reduce,
a 7-step cross-partition max fold, and a tiny host-side affine fixup.
"""

import sys

for _p in ("/opt/trn_rl_repo", "/opt/pypackages"):
    if _p not in sys.path:
        sys.path.append(_p)

import ml_dtypes
import numpy as np

import concourse.bass as bass
import concourse.tile as tile
from concourse import bacc, mybir
from concourse.bass_utils import run_bass_kernel_spmd

N, H, W = 100, 800, 1280
N_CORES = 8
K = 13  # masks per core (8 * 13 = 104 >= 100, zero-padded)
THRESHOLD = 0.5

fp32 = mybir.dt.float32
fp16 = mybir.dt.float16
bf16 = mybir.dt.bfloat16
Op = mybir.AluOpType


def _row_tiles(h):
    """[(row0, nrows), ...] covering h rows in 128-row tiles."""
    return [(r, min(128, h - r)) for r in range(0, h, 128)]


def _chunks(w):
    """[(col0, ncols), ...] covering w cols in <=512-col chunks (PSUM bank)."""
    return [(c, min(512, w - c)) for c in range(0, w, 512)]


def _ntp(h):
    """Columns per mask in the row-any tile: tile count padded to even."""
    nt = len(_row_tiles(h))
    return nt + (nt & 1)


def build_program(k=K, h=H, w=W):
    """One-core Bass/Tile program; run SPMD on all 8 cores."""
    tiles = _row_tiles(h)
    chunks = _chunks(w)
    ntp = _ntp(h)
    nt = len(tiles)

    nc = bacc.Bacc(
        "TRN2", target_bir_lowering=False, debug=False, enable_asserts=False
    )
    masks = nc.dram_tensor("masks", [k, h, w], fp32, kind="ExternalInput").ap()
    c1r = nc.dram_tensor("c1r", [128, ntp], fp16, kind="ExternalInput").ap()
    c2r = nc.dram_tensor("c2r", [128, ntp], fp16, kind="ExternalInput").ap()
    c1x = nc.dram_tensor("c1x", [k, w], fp16, kind="ExternalInput").ap()
    c2x = nc.dram_tensor("c2x", [k, w], fp16, kind="ExternalInput").ap()
    # ohs[p, c] = 1 iff c == k-1: sliding window ohs[:, k-1-j : 2k-1-j] is a
    # [128, k] one-hot-column matrix selecting PSUM output partition j.
    ohs = nc.dram_tensor("ohs", [128, 2 * k], bf16, kind="ExternalInput").ap()
    rows_out = nc.dram_tensor(
        "rows_out", [128, 2 * k], fp16, kind="ExternalOutput"
    ).ap()
    cols_out = nc.dram_tensor("cols_out", [k, 2], fp16, kind="ExternalOutput").ap()

    # PSUM accumulation is split into two mask groups so the first group's
    # cols reduction overlaps the second group's main loop instead of all
    # landing in a serial tail after the last matmul.
    g0 = (k + 1) // 2
    groups = [(0, g0)] + ([(g0, k - g0)] if k > g0 else [])
    # DMA row-tiles in pairs (1.25MB per transfer) for fewer, larger DMAs
    pairs = [(t, min(2, nt - t)) for t in range(0, nt, 2)]

    with tile.TileContext(nc) as tc:
        with (
            tc.tile_pool(name="raw", bufs=7) as rawp,
            tc.tile_pool(name="bin", bufs=8) as binp,
            tc.tile_pool(name="consts", bufs=1) as constp,
            tc.tile_pool(name="small", bufs=2) as smallp,
            tc.tile_pool(name="psum", bufs=1, space="PSUM") as psump,
        ):
            # consts ride gpsimd SWDGE queues so SP's HWDGE queues start
            # streaming mask tiles immediately
            c1r_t = constp.tile([128, ntp], fp16)
            nc.gpsimd.dma_start(c1r_t[:], c1r)
            c2r_t = constp.tile([128, ntp], fp16)
            nc.gpsimd.dma_start(c2r_t[:], c2r)
            c1x_t = constp.tile([k, w], fp16)
            nc.gpsimd.dma_start(c1x_t[:], c1x)
            c2x_t = constp.tile([k, w], fp16)
            nc.gpsimd.dma_start(c2x_t[:], c2x)
            ohs_t = constp.tile([128, 2 * k], bf16)
            nc.gpsimd.dma_start(ohs_t[:], ohs)

            rowany = constp.tile([128, k * ntp], fp32)
            nc.gpsimd.memset(rowany[:], 0.0)
            negh = constp.tile([128, 1], fp32)
            nc.gpsimd.memset(negh[:], -THRESHOLD)
            rr = constp.tile([128, 2 * k], fp16)
            cc = [
                [
                    psump.tile([gsz, cw], fp32, name=f"cc{g}_{ci}", tag=f"cc{g}_{ci}")
                    for ci, (_, cw) in enumerate(chunks)
                ]
                for g, (_, gsz) in enumerate(groups)
            ]

            def emit_cols_tail(g, gstart, gsz):
                """(count > 0) * iota per chunk, then max-reduce along X."""
                csc1 = constp.tile([gsz, w], fp16, name=f"csc1_{g}")
                csc2 = constp.tile([gsz, w], fp16, name=f"csc2_{g}")
                cr = constp.tile([gsz, 2], fp16, name=f"cr_{g}")
                for ci, (c0, cw) in enumerate(chunks):
                    nc.vector.scalar_tensor_tensor(
                        out=csc1[:, c0 : c0 + cw],
                        in0=cc[g][ci][:, :],
                        scalar=0.0,
                        in1=c1x_t[0:gsz, c0 : c0 + cw],
                        op0=Op.is_gt,
                        op1=Op.mult,
                    )
                    nc.vector.scalar_tensor_tensor(
                        out=csc2[:, c0 : c0 + cw],
                        in0=cc[g][ci][:, :],
                        scalar=0.0,
                        in1=c2x_t[0:gsz, c0 : c0 + cw],
                        op0=Op.is_gt,
                        op1=Op.mult,
                    )
                nc.vector.tensor_reduce(
                    out=cr[:, 0:1], in_=csc1[:], axis=mybir.AxisListType.X, op=Op.max
                )
                nc.vector.tensor_reduce(
                    out=cr[:, 1:2], in_=csc2[:], axis=mybir.AxisListType.X, op=Op.max
                )
                # gpsimd SWDGE: the in-order SP queue must not wait on cr
                nc.gpsimd.dma_start(cols_out[gstart : gstart + gsz, :], cr[:])

            for g, (gstart, gsz) in enumerate(groups):
                for jl in range(gsz):
                    j = gstart + jl
                    for tp, npair in pairs:
                        r0 = tiles[tp][0]
                        prows = sum(tiles[tp + i][1] for i in range(npair))
                        raw = rawp.tile([128, 2 * w], fp32)
                        if npair == 2 and prows == 256:
                            nc.sync.dma_start(
                                raw[:, : 2 * w].rearrange("p (a x) -> p a x", a=2),
                                masks[j, r0 : r0 + 256, :].rearrange(
                                    "(a p) x -> p a x", p=128
                                ),
                            )
                        else:
                            nc.sync.dma_start(
                                raw[: tiles[tp][1], :w],
                                masks[j, r0 : r0 + tiles[tp][1], :],
                            )
                            if npair == 2:
                                nc.sync.dma_start(
                                    raw[: tiles[tp + 1][1], w : 2 * w],
                                    masks[j, r0 + 128 : r0 + 128 + tiles[tp + 1][1], :],
                                )
                        for i in range(npair):
                            t = tp + i
                            nr = tiles[t][1]
                            rv = raw[:nr, i * w : i * w + w]
                            b = binp.tile([128, w], bf16)
                            acc = rowany[:nr, j * ntp + t : j * ntp + t + 1]
                            if (j * nt + t) % 2 == 0:
                                # DVE: binary = (x > 0.5), accum(max) = row-any
                                nc.vector.tensor_scalar(
                                    out=b[:nr, :],
                                    in0=rv,
                                    scalar1=THRESHOLD,
                                    scalar2=None,
                                    op0=Op.is_gt,
                                    op1=Op.max,
                                    accum_out=acc,
                                )
                            else:
                                # ACT: relu(x-0.5) > 0 iff x > 0.5 (exact);
                                # accum(sum of non-negatives) > 0 iff row-any
                                nc.scalar.activation(
                                    out=b[:nr, :],
                                    in_=rv,
                                    func=mybir.ActivationFunctionType.Relu,
                                    bias=negh[:nr, :],
                                    scale=1.0,
                                    accum_out=acc,
                                )
                            for ci, (c0, cw) in enumerate(chunks):
                                nc.tensor.matmul(
                                    cc[g][ci][:, :],
                                    ohs_t[:nr, k - 1 - jl : k - 1 - jl + gsz],
                                    b[:nr, c0 : c0 + cw],
                                    start=(jl == 0 and t == 0),
                                    stop=(jl == gsz - 1 and t == nt - 1),
                                )
                    # rows tail: rr[:, 2j] = max_t((any>0) * (H - idx)),
                    #            rr[:, 2j+1] = max_t((any>0) * (idx + 1))
                    sc1 = smallp.tile([128, ntp], fp16, tag="sc1")
                    nc.vector.scalar_tensor_tensor(
                        out=sc1[:],
                        in0=rowany[:, j * ntp : (j + 1) * ntp],
                        scalar=0.0,
                        in1=c1r_t[:],
                        op0=Op.is_gt,
                        op1=Op.mult,
                    )
                    nc.vector.tensor_reduce(
                        out=rr[:, 2 * j : 2 * j + 1],
                        in_=sc1[:],
                        axis=mybir.AxisListType.X,
                        op=Op.max,
                    )
                    sc2 = smallp.tile([128, ntp], fp16, tag="sc2")
                    nc.vector.scalar_tensor_tensor(
                        out=sc2[:],
                        in0=rowany[:, j * ntp : (j + 1) * ntp],
                        scalar=0.0,
                        in1=c2r_t[:],
                        op0=Op.is_gt,
                        op1=Op.mult,
                    )
                    nc.vector.tensor_reduce(
                        out=rr[:, 2 * j + 1 : 2 * j + 2],
                        in_=sc2[:],
                        axis=mybir.AxisListType.X,
                        op=Op.max,
                    )
                emit_cols_tail(g, gstart, gsz)

            # the 128-partition max fold of rr happens host-side (6.6KB/core)
            nc.gpsimd.dma_start(rows_out, rr[:])

    nc.compile()
    return nc


def make_consts(k=K, h=H, w=W):
    tiles = _row_tiles(h)
    ntp = _ntp(h)
    p = np.arange(128)

    c1r = np.zeros((128, ntp), np.float16)
    c2r = np.zeros((128, ntp), np.float16)
    for t, (r0, nr) in enumerate(tiles):
        idx = r0 + p
        valid = p < nr
        c1r[:, t] = np.where(valid, h - idx, 0)
        c2r[:, t] = np.where(valid, idx + 1, 0)

    x = np.arange(w)
    c1x = np.broadcast_to((w - x).astype(np.float16), (k, w)).copy()
    c2x = np.broadcast_to((x + 1).astype(np.float16), (k, w)).copy()
    ohs = np.zeros((128, 2 * k), ml_dtypes.bfloat16)
    ohs[:, k - 1] = 1
    return {"c1r": c1r, "c2r": c2r, "c1x": c1x, "c2x": c2x, "ohs": ohs}


def postprocess(results, k=K, h=H, w=W):
    """Per-core (rows_out, cols_out) -> boxes [n_cores * k, 2, 2] f32."""
    boxes = np.empty((len(results) * k, 2, 2), np.float32)
    for c, r in enumerate(results):
        rows = np.asarray(r["rows_out"], np.float32).max(axis=0)
        cols = np.asarray(r["cols_out"], np.float32).reshape(k, 2)
        sl = slice(c * k, (c + 1) * k)
        boxes[sl, 0, 0] = w - cols[:, 0]  # xmin
        boxes[sl, 0, 1] = h - rows[0::2]  # ymin
        boxes[sl, 1, 0] = cols[:, 1] - 1  # xmax
        boxes[sl, 1, 1] = rows[1::2] - 1  # ymax
    return boxes


_cache = {}


def _get_program():
    if "nc" not in _cache:
        _cache["nc"] = build_program()
        _cache["consts"] = make_consts()
    return _cache["nc"], _cache["consts"]


def make_in_maps(masks):
    """Pad masks to 104 and build the 8 per-core input maps."""
    masks = np.ascontiguousarray(np.asarray(masks, dtype=np.float32))
    _, consts = _get_program()
    n_pad = N_CORES * K
    if masks.shape[0] < n_pad:
        pad = np.zeros((n_pad - masks.shape[0], H, W), np.float32)
        masks = np.concatenate([masks, pad], axis=0)
    return [
        {"masks": masks[c * K : (c + 1) * K], **consts} for c in range(N_CORES)
    ]


def kernel(masks):
    nc, _ = _get_program()
    in_maps = make_in_maps(masks)
    res = run_bass_kernel_spmd(nc, in_maps, core_ids=list(range(N_CORES)))
    return postprocess(res.results)[:N]
